# revision 10
# baseline (speedup 1.0000x reference)
"""EyesMouthLoss Trainium2 kernel.

loss = mean(|pred-target| * (1 + 299*clip(eye_mask+mouth_mask, 0, 1)))

Sharding: pure data-parallel over B=16 -> 2 batches per core on 8 cores.
Host sums the 8 per-core partial scalars (the final all-reduce).

The masks depend only on `landmarks` (tiny: 16x68x2 ints), so the host
precomputes the priority field, quantizes it to u8 (256 KB/core next to
the 12.6 MB/core of fp32 pred/target), and ACT dequantizes it to the
bf16 weight w = 1 + (299/255)*w' once per chunk.

All loads ride the two HWDGE FIFOs (pred on SP, targ on ACT): HWDGE
avoids the SWDGE descriptor-ring port contention that makes SDMA
engines 7/15 straggle (which delays every SWDGE completion semaphore by
a growing ~15%), and its completion semaphores fire ~2 us after the
data lands.  fp32 staging rings (depth 6) keep the FIFOs ahead of
compute; the fp32->bf16 cast happens inside the subtract:

    d   = pred - target     DVE/Pool alternating, fp32 ins -> bf16 out
    a   = |d|               ACT Abs (bf16), fp32 accum_out (row |d| sum)
    g   = a * w             DVE scalar_tensor_tensor, w broadcast over
                            channels, fp32 accum_out = weighted row-sum

The last unit is split into x-halves so the tail chain after the final
DMA is short.  The [128, 36] fp32 accumulator tile is the only output;
the host applies the final 1/N while summing the 8 per-core partials.
"""

import sys

sys.path.insert(0, "/opt/trn_rl_repo")

from contextlib import ExitStack

import numpy as np

import concourse.bass as bass
import concourse.tile as tile
from concourse import bacc, mybir
from concourse.bass_utils import run_bass_kernel_spmd

B, C, H, W = 16, 3, 512, 512
NCORES = 8
BPC = B // NCORES  # batches per core
NCHUNK = 4  # 512 rows = 4 x 128 partitions
NSTAGE = 8  # fp32 staging depth — every unit has its own slot (no WAR)
RADIUS = 15.0
HALF = 14  # region strictly zero for |dx| >= 15
EYE = (36, 48)
MOUTH = (48, 68)
WEIGHT = 300.0
NTOT = float(B * C * H * W)
FP32 = mybir.dt.float32
BF16 = mybir.dt.bfloat16
U8 = mybir.dt.uint8
Alu = mybir.AluOpType
Act = mybir.ActivationFunctionType

_STENCIL = None


def _stencil():
    global _STENCIL
    if _STENCIL is None:
        d = np.arange(2 * HALF + 1, dtype=np.float32) - HALF
        r = np.sqrt(d[:, None] ** 2 + d[None, :] ** 2)
        _STENCIL = np.clip(1.0 - r / RADIUS, 0.0, 1.0).astype(np.float32)
    return _STENCIL


def _priority_u8(landmarks):
    """w'[b,y,x] = round(255*clip(eye+mouth, 0, 1)), computed on host."""
    st = _stencil()
    w = np.empty((B, H, W), np.uint8)
    for b in range(B):
        fields = np.zeros((2, H, W), np.float32)
        for field, (lo, hi) in zip(fields, (EYE, MOUTH)):
            for cx, cy in landmarks[b, lo:hi]:
                cx = int(min(max(int(cx), 0), W - 1))
                cy = int(min(max(int(cy), 0), H - 1))
                y0, y1 = max(0, cy - HALF), min(H - 1, cy + HALF)
                x0, x1 = max(0, cx - HALF), min(W - 1, cx + HALF)
                sy0, sx0 = y0 - (cy - HALF), x0 - (cx - HALF)
                np.maximum(
                    field[y0 : y1 + 1, x0 : x1 + 1],
                    st[sy0 : sy0 + y1 - y0 + 1, sx0 : sx0 + x1 - x0 + 1],
                    out=field[y0 : y1 + 1, x0 : x1 + 1],
                )
        w[b] = np.rint(
            255.0 * np.minimum(fields[0] + fields[1], 1.0)
        ).astype(np.uint8)
    return w


def _build():
    """Build the SPMD Bass program (shared by all cores; data-parallel)."""
    nc = bacc.Bacc(None)
    pred_p = nc.declare_dram_parameter("pred", [BPC, C, H, W], FP32, isOutput=False)
    targ_p = nc.declare_dram_parameter("targ", [BPC, C, H, W], FP32, isOutput=False)
    wgt_p = nc.declare_dram_parameter("wgt", [BPC, NCHUNK, 128, W], U8, isOutput=False)
    nu = BPC * NCHUNK
    nacc = 2 * nu + 4  # [abs sums | weighted sums], last unit split in two
    out_p = nc.declare_dram_parameter("out", [128, nacc], FP32, isOutput=True)

    with tile.TileContext(nc) as tc, ExitStack() as ctx:
        stat_pool = ctx.enter_context(tc.tile_pool(name="stat", bufs=2))
        load_pool = ctx.enter_context(tc.tile_pool(name="load", bufs=2))

        units = [(bi, k) for bi in range(BPC) for k in range(NCHUNK)]
        rs = stat_pool.tile([128, nacc], FP32)

        w_u8 = load_pool.tile([128, BPC, NCHUNK, W], U8, tag="w_u8")
        stage_p = [
            load_pool.tile([128, C, W], FP32, tag="sp", name=f"sp{s}")
            for s in range(NSTAGE)
        ]
        stage_t = [
            load_pool.tile([128, C, W], FP32, tag="st", name=f"st{s}")
            for s in range(NSTAGE)
        ]
        d_t = [
            load_pool.tile([128, C, W], BF16, tag="d", name=f"d{s}")
            for s in range(NSTAGE)
        ]
        a_t = [
            load_pool.tile([128, C, W], BF16, tag="a", name=f"a{s}")
            for s in range(NSTAGE)
        ]
        w_e = [
            load_pool.tile([128, W], BF16, tag="we", name=f"we{s}")
            for s in range(NSTAGE)
        ]

        # xh: optional x-slice (for the split last unit)
        def xsl(xh):
            return slice(None) if xh is None else slice(xh * (W // 2), (xh + 1) * (W // 2))

        def load(u, xh=None):
            bi, k = units[u]
            s = u % NSTAGE
            rows = slice(128 * k, 128 * (k + 1))
            nc.sync.dma_start(
                stage_p[s][:, :, xsl(xh)],
                pred_p[bi, :, rows, xsl(xh)].rearrange("c p x -> p c x"),
            )
            nc.scalar.dma_start(
                stage_t[s][:, :, xsl(xh)],
                targ_p[bi, :, rows, xsl(xh)].rearrange("c p x -> p c x"),
            )

        def wexp(u):
            bi, k = units[u]
            s = u % NSTAGE
            nc.scalar.activation(
                w_e[s][:], w_u8[:, bi, k, :], Act.Identity,
                bias=1.0, scale=(WEIGHT - 1.0) / 255.0,
            )

        def sub(u, xh=None, engine=None):
            s = u % NSTAGE
            engine = engine or nc.vector
            engine.tensor_tensor(
                d_t[s][:, :, xsl(xh)], stage_p[s][:, :, xsl(xh)],
                stage_t[s][:, :, xsl(xh)], op=Alu.subtract,
            )

        def abs_(u, xh=None, col=None):
            s = u % NSTAGE
            nc.scalar.activation(
                a_t[s][:, :, xsl(xh)], d_t[s][:, :, xsl(xh)], Act.Abs,
                accum_out=rs[:, col : col + 1],
            )

        def stt(u, xh=None, col=None):
            s = u % NSTAGE
            xs = xsl(xh)
            wn = W if xh is None else W // 2
            wb = (
                w_e[s][:, xs]
                .broadcast_to([128, wn, C])
                .rearrange("p x c -> p c x")
            )
            nc.vector.scalar_tensor_tensor(
                d_t[s][:, :, xs], a_t[s][:, :, xs], 1.0, wb,
                op0=Alu.mult, op1=Alu.mult,
                accum_out=rs[:, col : col + 1],
            )

        # ---- emission ----
        last = nu - 1
        load(0)
        # w' after the first unit's loads: tiny, needed only by stt(0)
        for bi in range(BPC):
            nc.sync.dma_start(
                w_u8[:, bi, :, :], wgt_p[bi].rearrange("k p x -> p k x")
            )
        for u in range(1, NSTAGE):
            load(u)
        wexp(0)
        sub(0)
        abs_(0, col=0)
        for u in range(1, nu):
            nxt = u + NSTAGE - 1
            if nxt < last:
                load(nxt)
            elif nxt == last:
                load(last, xh=0)
                load(last, xh=1)
            if u < last:
                wexp(u)
                sub(u)
                abs_(u, col=u)
                stt(u - 1, col=nu + u - 1)
            else:  # split last unit into x-halves
                wexp(u)
                stt(u - 1, col=nu + u - 1)
                for xh in (0, 1):
                    sub(u, xh=xh, engine=nc.vector)
                    abs_(u, xh=xh, col=2 * nu + xh)
                    stt(u, xh=xh, col=2 * nu + 2 + xh)

        nc.sync.dma_start(out_p[:, :], rs[:])

    return nc


def run(inputs, trace=False):
    pred = np.ascontiguousarray(inputs["pred"], dtype=np.float32)
    targ = np.ascontiguousarray(inputs["target"], dtype=np.float32)
    lms = np.asarray(inputs["landmarks"])
    assert pred.shape == (B, C, H, W) and targ.shape == (B, C, H, W)

    w = _priority_u8(lms).reshape(B, NCHUNK, 128, W)

    nc = _build()
    nc.finalize()
    in_maps = [
        {
            "pred": pred[i * BPC : (i + 1) * BPC],
            "targ": targ[i * BPC : (i + 1) * BPC],
            "wgt": w[i * BPC : (i + 1) * BPC],
        }
        for i in range(NCORES)
    ]
    res = run_bass_kernel_spmd(nc, in_maps, list(range(NCORES)), trace=trace)
    nu = BPC * NCHUNK
    total = 0.0
    for i in range(NCORES):
        part = res.results[i]["out"].astype(np.float64)
        # weighted sums: cols [nu, 2nu) for units 0..nu-2, plus the split
        # last unit's halves at [2nu+2, 2nu+4)
        total += part[:, nu : 2 * nu - 1].sum() + part[:, 2 * nu + 2 :].sum()
    return np.float32(total / NTOT), res


def kernel(pred, target, landmarks):
    out, _ = run({"pred": pred, "target": target, "landmarks": landmarks})
    return out


# revision 11
# speedup vs baseline: 1.0256x; 1.0256x over previous
"""EyesMouthLoss Trainium2 kernel.

loss = mean(|pred-target| * (1 + 299*clip(eye_mask+mouth_mask, 0, 1)))

Sharding: pure data-parallel over B=16 -> 2 batches per core on 8 cores.
Host sums the 8 per-core partial scalars (the final all-reduce).

The masks depend only on `landmarks` (tiny: 16x68x2 ints), so the host
precomputes the priority field, quantizes it to u8 (256 KB/core next to
the 12.6 MB/core of fp32 pred/target), and ACT dequantizes it to the
bf16 weight w = 1 + (299/255)*w' once per chunk.

All loads ride the two HWDGE FIFOs (pred on SP, targ on ACT): HWDGE
avoids the SWDGE descriptor-ring port contention that makes SDMA
engines 7/15 straggle (which delays every SWDGE completion semaphore by
a growing ~15%), and its completion semaphores fire ~2 us after the
data lands.  fp32 staging rings (depth 6) keep the FIFOs ahead of
compute; the fp32->bf16 cast happens inside the subtract:

    d   = pred - target     DVE/Pool alternating, fp32 ins -> bf16 out
    a   = |d|               ACT Abs (bf16), fp32 accum_out (row |d| sum)
    g   = a * w             DVE scalar_tensor_tensor, w broadcast over
                            channels, fp32 accum_out = weighted row-sum

The last unit is split into x-halves so the tail chain after the final
DMA is short.  The [128, 36] fp32 accumulator tile is the only output;
the host applies the final 1/N while summing the 8 per-core partials.
"""

import sys

sys.path.insert(0, "/opt/trn_rl_repo")

from contextlib import ExitStack

import numpy as np

import concourse.bass as bass
import concourse.tile as tile
from concourse import bacc, mybir
from concourse.bass_utils import run_bass_kernel_spmd

B, C, H, W = 16, 3, 512, 512
NCORES = 8
BPC = B // NCORES  # batches per core
NCHUNK = 4  # 512 rows = 4 x 128 partitions
NSTAGE = 8  # fp32 staging depth — every unit has its own slot (no WAR)
RADIUS = 15.0
HALF = 14  # region strictly zero for |dx| >= 15
EYE = (36, 48)
MOUTH = (48, 68)
WEIGHT = 300.0
NTOT = float(B * C * H * W)
FP32 = mybir.dt.float32
BF16 = mybir.dt.bfloat16
U8 = mybir.dt.uint8
Alu = mybir.AluOpType
Act = mybir.ActivationFunctionType

_STENCIL = None


def _stencil():
    global _STENCIL
    if _STENCIL is None:
        d = np.arange(2 * HALF + 1, dtype=np.float32) - HALF
        r = np.sqrt(d[:, None] ** 2 + d[None, :] ** 2)
        _STENCIL = np.clip(1.0 - r / RADIUS, 0.0, 1.0).astype(np.float32)
    return _STENCIL


def _priority_u8(landmarks):
    """w'[b,y,x] = round(255*clip(eye+mouth, 0, 1)), computed on host."""
    st = _stencil()
    w = np.empty((B, H, W), np.uint8)
    for b in range(B):
        fields = np.zeros((2, H, W), np.float32)
        for field, (lo, hi) in zip(fields, (EYE, MOUTH)):
            for cx, cy in landmarks[b, lo:hi]:
                cx = int(min(max(int(cx), 0), W - 1))
                cy = int(min(max(int(cy), 0), H - 1))
                y0, y1 = max(0, cy - HALF), min(H - 1, cy + HALF)
                x0, x1 = max(0, cx - HALF), min(W - 1, cx + HALF)
                sy0, sx0 = y0 - (cy - HALF), x0 - (cx - HALF)
                np.maximum(
                    field[y0 : y1 + 1, x0 : x1 + 1],
                    st[sy0 : sy0 + y1 - y0 + 1, sx0 : sx0 + x1 - x0 + 1],
                    out=field[y0 : y1 + 1, x0 : x1 + 1],
                )
        w[b] = np.rint(
            255.0 * np.minimum(fields[0] + fields[1], 1.0)
        ).astype(np.uint8)
    return w


def _build():
    """Build the SPMD Bass program (shared by all cores; data-parallel)."""
    nc = bacc.Bacc(None)
    pred_p = nc.declare_dram_parameter("pred", [BPC, C, H, W], FP32, isOutput=False)
    targ_p = nc.declare_dram_parameter("targ", [BPC, C, H, W], FP32, isOutput=False)
    wgt_p = nc.declare_dram_parameter("wgt", [BPC, NCHUNK, 128, W], U8, isOutput=False)
    nu = BPC * NCHUNK
    nacc = 2 * nu + 4  # [abs sums | weighted sums], last unit split in two
    out_p = nc.declare_dram_parameter("out", [128, nacc], FP32, isOutput=True)

    with tile.TileContext(nc) as tc, ExitStack() as ctx:
        stat_pool = ctx.enter_context(tc.tile_pool(name="stat", bufs=2))
        load_pool = ctx.enter_context(tc.tile_pool(name="load", bufs=2))

        units = [(bi, k) for bi in range(BPC) for k in range(NCHUNK)]
        rs = stat_pool.tile([128, nacc], FP32)

        w_u8 = load_pool.tile([128, BPC, NCHUNK, W], U8, tag="w_u8")
        stage_p = [
            load_pool.tile([128, C, W], FP32, tag="sp", name=f"sp{s}")
            for s in range(NSTAGE)
        ]
        stage_t = [
            load_pool.tile([128, C, W], FP32, tag="st", name=f"st{s}")
            for s in range(NSTAGE)
        ]
        d_t = [
            load_pool.tile([128, C, W], BF16, tag="d", name=f"d{s}")
            for s in range(NSTAGE)
        ]
        a_t = [
            load_pool.tile([128, C, W], BF16, tag="a", name=f"a{s}")
            for s in range(NSTAGE)
        ]
        w_e = [
            load_pool.tile([128, W], BF16, tag="we", name=f"we{s}")
            for s in range(NSTAGE)
        ]

        # xh: optional x-slice (for the split last unit)
        def xsl(xh):
            return slice(None) if xh is None else slice(xh * (W // 2), (xh + 1) * (W // 2))

        def load(u, xh=None):
            bi, k = units[u]
            s = u % NSTAGE
            rows = slice(128 * k, 128 * (k + 1))
            nc.sync.dma_start(
                stage_p[s][:, :, xsl(xh)],
                pred_p[bi, :, rows, xsl(xh)].rearrange("c p x -> p c x"),
            )
            nc.sync.dma_start(
                stage_t[s][:, :, xsl(xh)],
                targ_p[bi, :, rows, xsl(xh)].rearrange("c p x -> p c x"),
            )

        def wexp(u):
            bi, k = units[u]
            s = u % NSTAGE
            nc.scalar.activation(
                w_e[s][:], w_u8[:, bi, k, :], Act.Identity,
                bias=1.0, scale=(WEIGHT - 1.0) / 255.0,
            )

        def sub(u, xh=None, engine=None):
            s = u % NSTAGE
            engine = engine or nc.vector
            engine.tensor_tensor(
                d_t[s][:, :, xsl(xh)], stage_p[s][:, :, xsl(xh)],
                stage_t[s][:, :, xsl(xh)], op=Alu.subtract,
            )

        def abs_(u, xh=None, col=None):
            s = u % NSTAGE
            nc.scalar.activation(
                a_t[s][:, :, xsl(xh)], d_t[s][:, :, xsl(xh)], Act.Abs,
                accum_out=rs[:, col : col + 1],
            )

        def stt(u, xh=None, col=None):
            s = u % NSTAGE
            xs = xsl(xh)
            wn = W if xh is None else W // 2
            wb = (
                w_e[s][:, xs]
                .broadcast_to([128, wn, C])
                .rearrange("p x c -> p c x")
            )
            nc.vector.scalar_tensor_tensor(
                d_t[s][:, :, xs], a_t[s][:, :, xs], 1.0, wb,
                op0=Alu.mult, op1=Alu.mult,
                accum_out=rs[:, col : col + 1],
            )

        # ---- emission ----
        last = nu - 1
        load(0)
        # w' after the first unit's loads: tiny, needed only by stt(0)
        for bi in range(BPC):
            nc.sync.dma_start(
                w_u8[:, bi, :, :], wgt_p[bi].rearrange("k p x -> p k x")
            )
        for u in range(1, NSTAGE):
            load(u)
        wexp(0)
        sub(0)
        abs_(0, col=0)
        for u in range(1, nu):
            nxt = u + NSTAGE - 1
            if nxt < last:
                load(nxt)
            elif nxt == last:
                load(last, xh=0)
                load(last, xh=1)
            if u < last:
                wexp(u)
                sub(u)
                abs_(u, col=u)
                stt(u - 1, col=nu + u - 1)
            else:  # split last unit into x-halves
                wexp(u)
                stt(u - 1, col=nu + u - 1)
                for xh in (0, 1):
                    sub(u, xh=xh, engine=nc.vector)
                    abs_(u, xh=xh, col=2 * nu + xh)
                    stt(u, xh=xh, col=2 * nu + 2 + xh)

        nc.sync.dma_start(out_p[:, :], rs[:])

    return nc


def run(inputs, trace=False):
    pred = np.ascontiguousarray(inputs["pred"], dtype=np.float32)
    targ = np.ascontiguousarray(inputs["target"], dtype=np.float32)
    lms = np.asarray(inputs["landmarks"])
    assert pred.shape == (B, C, H, W) and targ.shape == (B, C, H, W)

    w = _priority_u8(lms).reshape(B, NCHUNK, 128, W)

    nc = _build()
    nc.finalize()
    in_maps = [
        {
            "pred": pred[i * BPC : (i + 1) * BPC],
            "targ": targ[i * BPC : (i + 1) * BPC],
            "wgt": w[i * BPC : (i + 1) * BPC],
        }
        for i in range(NCORES)
    ]
    res = run_bass_kernel_spmd(nc, in_maps, list(range(NCORES)), trace=trace)
    nu = BPC * NCHUNK
    total = 0.0
    for i in range(NCORES):
        part = res.results[i]["out"].astype(np.float64)
        # weighted sums: cols [nu, 2nu) for units 0..nu-2, plus the split
        # last unit's halves at [2nu+2, 2nu+4)
        total += part[:, nu : 2 * nu - 1].sum() + part[:, 2 * nu + 2 :].sum()
    return np.float32(total / NTOT), res


def kernel(pred, target, landmarks):
    out, _ = run({"pred": pred, "target": target, "landmarks": landmarks})
    return out


# revision 12
# speedup vs baseline: 1.0893x; 1.0621x over previous
"""EyesMouthLoss Trainium2 kernel.

loss = mean(|pred-target| * (1 + 299*clip(eye_mask+mouth_mask, 0, 1)))

Sharding: pure data-parallel over B=16 -> 2 batches per core on 8 cores.
Host sums the 8 per-core partial scalars (the final all-reduce).

The masks depend only on `landmarks` (tiny: 16x68x2 ints), so the host
precomputes the priority field, quantizes it to u8 (256 KB/core next to
the 12.6 MB/core of fp32 pred/target), and ACT dequantizes it to the
bf16 weight w = 1 + (299/255)*w' once per chunk.

pred/target stream through SWDGE casting DMAs (fp32 HBM -> bf16 SBUF on
gpsimd; SWDGE sustains ~400-420 GB/s vs ~320 for an HWDGE ring).  Per
128-row chunk the compute is a 3-op bf16 stream:

    d   = pred - target     DVE tensor_tensor (bf16, full rate)
    a   = |d|               ACT Abs, fp32 accum_out (row |d| sums)
    g   = a * w             DVE scalar_tensor_tensor, w broadcast over
                            channels, fp32 accum_out = weighted row-sum

The STT is emitted one unit behind its SUB so the DVE queue head never
waits on the cross-engine ABS; the last two units are split into
x-halves (loads and compute) so the tail chain after the final DMA
completion is short.  The [128, 2*8+8] fp32 accumulator tile is the
only output; the host applies the final 1/N over the 8 cores.
"""

import sys

sys.path.insert(0, "/opt/trn_rl_repo")

from contextlib import ExitStack

import numpy as np

import concourse.bass as bass
import concourse.tile as tile
from concourse import bacc, mybir
from concourse.bass_utils import run_bass_kernel_spmd

B, C, H, W = 16, 3, 512, 512
NCORES = 8
BPC = B // NCORES  # batches per core
NCHUNK = 4  # 512 rows = 4 x 128 partitions
NSPLIT = 2  # trailing units computed in x-halves for a short tail
RADIUS = 15.0
HALF = 14  # region strictly zero for |dx| >= 15
EYE = (36, 48)
MOUTH = (48, 68)
WEIGHT = 300.0
NTOT = float(B * C * H * W)
FP32 = mybir.dt.float32
BF16 = mybir.dt.bfloat16
U8 = mybir.dt.uint8
Alu = mybir.AluOpType
Act = mybir.ActivationFunctionType

_STENCIL = None


def _stencil():
    global _STENCIL
    if _STENCIL is None:
        d = np.arange(2 * HALF + 1, dtype=np.float32) - HALF
        r = np.sqrt(d[:, None] ** 2 + d[None, :] ** 2)
        _STENCIL = np.clip(1.0 - r / RADIUS, 0.0, 1.0).astype(np.float32)
    return _STENCIL


def _priority_u8(landmarks):
    """w'[b,y,x] = round(255*clip(eye+mouth, 0, 1)), computed on host."""
    st = _stencil()
    w = np.empty((B, H, W), np.uint8)
    for b in range(B):
        fields = np.zeros((2, H, W), np.float32)
        for field, (lo, hi) in zip(fields, (EYE, MOUTH)):
            for cx, cy in landmarks[b, lo:hi]:
                cx = int(min(max(int(cx), 0), W - 1))
                cy = int(min(max(int(cy), 0), H - 1))
                y0, y1 = max(0, cy - HALF), min(H - 1, cy + HALF)
                x0, x1 = max(0, cx - HALF), min(W - 1, cx + HALF)
                sy0, sx0 = y0 - (cy - HALF), x0 - (cx - HALF)
                np.maximum(
                    field[y0 : y1 + 1, x0 : x1 + 1],
                    st[sy0 : sy0 + y1 - y0 + 1, sx0 : sx0 + x1 - x0 + 1],
                    out=field[y0 : y1 + 1, x0 : x1 + 1],
                )
        w[b] = np.rint(
            255.0 * np.minimum(fields[0] + fields[1], 1.0)
        ).astype(np.uint8)
    return w


def _build():
    """Build the SPMD Bass program (shared by all cores; data-parallel)."""
    nc = bacc.Bacc(None)
    pred_p = nc.declare_dram_parameter("pred", [BPC, C, H, W], FP32, isOutput=False)
    targ_p = nc.declare_dram_parameter("targ", [BPC, C, H, W], FP32, isOutput=False)
    wgt_p = nc.declare_dram_parameter("wgt", [BPC, NCHUNK, 128, W], U8, isOutput=False)
    nu = BPC * NCHUNK
    nacc = 2 * nu + 4 * NSPLIT
    out_p = nc.declare_dram_parameter("out", [128, nacc], FP32, isOutput=True)

    with tile.TileContext(nc) as tc, ExitStack() as ctx:
        stat_pool = ctx.enter_context(tc.tile_pool(name="stat", bufs=2))
        load_pool = ctx.enter_context(tc.tile_pool(name="load", bufs=2))

        units = [(bi, k) for bi in range(BPC) for k in range(NCHUNK)]
        rs = stat_pool.tile([128, nacc], FP32)

        w_u8 = load_pool.tile([128, BPC, NCHUNK, W], U8, tag="w_u8")
        w_e = load_pool.tile([128, BPC, NCHUNK, W], BF16, tag="w_e")
        p_ts, t_ts = [], []
        for bi in range(BPC):
            p_ts.append(load_pool.tile([128, C, NCHUNK, W], BF16, tag="p_t",
                                       name=f"p_t{bi}"))
            t_ts.append(load_pool.tile([128, C, NCHUNK, W], BF16, tag="t_t",
                                       name=f"t_t{bi}"))

        def xsl(xh):
            if xh is None:
                return slice(None)
            return slice(xh * (W // 2), (xh + 1) * (W // 2))

        def load(u, xh=None):
            bi, k = units[u]
            rows = slice(128 * k, 128 * (k + 1))
            nc.gpsimd.dma_start(
                p_ts[bi][:, :, k, xsl(xh)],
                pred_p[bi, :, rows, xsl(xh)].rearrange("c p x -> p c x"),
            )
            nc.gpsimd.dma_start(
                t_ts[bi][:, :, k, xsl(xh)],
                targ_p[bi, :, rows, xsl(xh)].rearrange("c p x -> p c x"),
            )

        def wexp(u):
            bi, k = units[u]
            nc.scalar.activation(
                w_e[:, bi, k, :], w_u8[:, bi, k, :], Act.Identity,
                bias=1.0, scale=(WEIGHT - 1.0) / 255.0,
            )

        def sub(u, xh=None):
            bi, k = units[u]
            nc.vector.tensor_tensor(
                p_ts[bi][:, :, k, xsl(xh)], p_ts[bi][:, :, k, xsl(xh)],
                t_ts[bi][:, :, k, xsl(xh)], op=Alu.subtract,
            )

        def abs_(u, xh=None, col=0):
            bi, k = units[u]
            nc.scalar.activation(
                t_ts[bi][:, :, k, xsl(xh)], p_ts[bi][:, :, k, xsl(xh)], Act.Abs,
                accum_out=rs[:, col : col + 1],
            )

        def stt(u, xh=None, col=0):
            bi, k = units[u]
            xs = xsl(xh)
            wn = W if xh is None else W // 2
            wb = (
                w_e[:, bi, k, xs]
                .broadcast_to([128, wn, C])
                .rearrange("p x c -> p c x")
            )
            nc.vector.scalar_tensor_tensor(
                p_ts[bi][:, :, k, xs], t_ts[bi][:, :, k, xs], 1.0, wb,
                op0=Alu.mult, op1=Alu.mult,
                accum_out=rs[:, col : col + 1],
            )

        # ---- emission: loads first (SWDGE FIFO = arrival order), w' on the
        # idle SP HWDGE ring, then the software-pipelined compute stream ----
        nfull = nu - NSPLIT
        load(0)
        for bi in range(BPC):
            nc.sync.dma_start(
                w_u8[:, bi, :, :], wgt_p[bi].rearrange("k p x -> p k x")
            )
        for u in range(1, nu):
            if u < nfull:
                load(u)
            else:
                load(u, xh=0)
                load(u, xh=1)

        wexp(0)
        sub(0)
        abs_(0, col=0)
        for u in range(1, nfull):
            wexp(u)
            sub(u)
            abs_(u, col=u)
            stt(u - 1, col=nu + u - 1)
        stt(nfull - 1, col=nu + nfull - 1)
        for i, u in enumerate(range(nfull, nu)):
            wexp(u)
            for xh in (0, 1):
                c = 2 * nu + 4 * i + 2 * xh
                sub(u, xh=xh)
                abs_(u, xh=xh, col=c)
                stt(u, xh=xh, col=c + 1)

        nc.sync.dma_start(out_p[:, :], rs[:])

    return nc


def run(inputs, trace=False):
    pred = np.ascontiguousarray(inputs["pred"], dtype=np.float32)
    targ = np.ascontiguousarray(inputs["target"], dtype=np.float32)
    lms = np.asarray(inputs["landmarks"])
    assert pred.shape == (B, C, H, W) and targ.shape == (B, C, H, W)

    w = _priority_u8(lms).reshape(B, NCHUNK, 128, W)

    nc = _build()
    nc.finalize()
    in_maps = [
        {
            "pred": pred[i * BPC : (i + 1) * BPC],
            "targ": targ[i * BPC : (i + 1) * BPC],
            "wgt": w[i * BPC : (i + 1) * BPC],
        }
        for i in range(NCORES)
    ]
    res = run_bass_kernel_spmd(nc, in_maps, list(range(NCORES)), trace=trace)
    nu = BPC * NCHUNK
    nfull = nu - NSPLIT
    total = 0.0
    for i in range(NCORES):
        part = res.results[i]["out"].astype(np.float64)
        # weighted sums: cols [nu, nu+nfull) for full units; for split units
        # the stt accums are at 2nu + 4i + {1, 3}
        total += part[:, nu : nu + nfull].sum()
        for j in range(NSPLIT):
            total += part[:, 2 * nu + 4 * j + 1] .sum()
            total += part[:, 2 * nu + 4 * j + 3].sum()
    return np.float32(total / NTOT), res


def kernel(pred, target, landmarks):
    out, _ = run({"pred": pred, "target": target, "landmarks": landmarks})
    return out
